# revision 2
# baseline (speedup 1.0000x reference)
"""CapsuleLayer dynamic-routing kernel, N-sharded across 8 Trainium2 cores.

Sharding: route-nodes N=1152 split 144/core (full batch per core), so the
N x KO - sized work (s-matmul streaming, Wc build, W*P, reductions) is
genuinely divided by 8 (the batch-parallel variant replicates it).

Per-core layout: r = n_sub*128 + (i*16 + n_off), 9 tiles of 128
partitions; partition p <-> (i = p//16, n_off = p%16), n_loc =
n_sub*16 + n_off, global n = core*144 + n_loc.  This makes the two
awkward reductions cheap constant 0/1 indicator matmuls on the PE:
  - i-reduce:  a[n_off, n_sub, k] = sum_p E2[p, n_off] * q[p, n_sub, k]
  - c-bcast :  c_rep[p, ...] = sum_j Erep[j, p] * c[j, ...]

Engine balance (all four compute engines loaded):
  - PE: s-matmul with per-iteration precision (iter0 bf16 hi/lo 3-chain,
    iter1 fp32, iter2 single bf16 -- error amplification through the
    b_ij feedback is ~150x for iter0 noise but 1x for iter2), P-matmul
    bf16 hi/lo 3-chain (K=256), indicator matmuls.
  - DVE: W*P multiply (PSUM read), squash via fused
    scalar_tensor_tensor divide trick (t=(sq+1)/sq; v=sg/t), softmax.
  - ACT: Square/Sign, PSUM evacuations, vh bf16 cast.
  - GPSIMD: o-reduce, Wc builds (SBUF-only), collective triggers.
  - Collectives: AllReduce s [256,160] per iter (hidden behind compute
    across in-NEFF repeats), final iter ReduceScatter only.
"""

import numpy as np

B, N, C, O, I = 256, 1152, 10, 16, 8
NCORES = 8
NL = N // NCORES      # 144 nodes per core
NSUB = 9              # 128-row r-tiles per core
KO = C * O            # 160
ITERS = 3

_BUILT = {}


def _build_program(num_devices=NCORES, collective=True, repeat=1):
    import concourse.bass as bass
    import concourse.mybir as mybir
    import concourse.tile as tile
    import concourse.bacc as bacc

    f32 = mybir.dt.float32
    bf16 = mybir.dt.bfloat16
    AX = mybir.AxisListType
    ALU = mybir.AluOpType
    ACT = mybir.ActivationFunctionType

    nc = bacc.Bacc("TRN2", target_bir_lowering=False, debug=False,
                   num_devices=num_devices)

    if repeat > 1:
        nc.dram_tensor("rep_tag", [1, repeat], f32, kind="ExternalInput")
    x_s_d = nc.dram_tensor("x_s", [128, NSUB * 256], f32, kind="ExternalInput")
    x_sh_d = nc.dram_tensor("x_sh", [128, NSUB * 256], bf16,
                            kind="ExternalInput")
    x_sl_d = nc.dram_tensor("x_sl", [128, NSUB * 256], bf16,
                            kind="ExternalInput")
    x_ph_d = nc.dram_tensor("x_ph", [128, 2 * NSUB * 128], bf16,
                            kind="ExternalInput")
    x_pl_d = nc.dram_tensor("x_pl", [128, 2 * NSUB * 128], bf16,
                            kind="ExternalInput")
    w_d = nc.dram_tensor("w_s", [128, NSUB * KO], f32, kind="ExternalInput")
    w0h_d = nc.dram_tensor("w0h", [128, NSUB * KO], bf16,
                           kind="ExternalInput")
    w0l_d = nc.dram_tensor("w0l", [128, NSUB * KO], bf16,
                           kind="ExternalInput")
    e2_d = nc.dram_tensor("e2", [128, 16], f32, kind="ExternalInput")
    erep_d = nc.dram_tensor("erep", [16, 128], f32, kind="ExternalInput")
    v_d = nc.dram_tensor("v_out", [16, 2 * KO], f32, kind="ExternalOutput")

    with tile.TileContext(nc) as tc:
        with (
            tc.tile_pool(name="main", bufs=1) as pool,
            tc.tile_pool(name="ps", bufs=2, space="PSUM") as ps,
            tc.tile_pool(name="pp", bufs=1, space="PSUM") as pp,
            tc.tile_pool(name="pa", bufs=1, space="PSUM") as pa,
            tc.tile_pool(name="dram", bufs=2, space="DRAM") as dram,
        ):
            x_s = pool.tile([128, NSUB * 256], f32)
            x_sh = pool.tile([128, NSUB * 256], bf16)
            x_sl = pool.tile([128, NSUB * 256], bf16)
            x_ph = pool.tile([128, 2 * NSUB * 128], bf16)
            x_pl = pool.tile([128, 2 * NSUB * 128], bf16)
            w_sb = pool.tile([128, NSUB * KO], f32)
            w0h = pool.tile([128, NSUB * KO], bf16)
            w0l = pool.tile([128, NSUB * KO], bf16)
            wc = pool.tile([128, NSUB * KO], f32)
            wc2h = pool.tile([128, NSUB * KO], bf16)
            wc2l = pool.tile([128, NSUB * KO], bf16)
            wp = pool.tile([128, NSUB * KO], f32)
            e2 = pool.tile([128, 16], f32)
            erep = pool.tile([16, 128], f32)
            b16 = pool.tile([16, NSUB * C], f32)
            q_sb = pool.tile([128, NSUB * C], f32)

            nc.sync.dma_start(e2[:, :], e2_d[:, :])
            nc.sync.dma_start(erep[:, :], erep_d[:, :])
            # interleave so iter-0 s-matmul tiles can start early
            for t in range(NSUB):
                nc.sync.dma_start(x_sh[:, t * 256:(t + 1) * 256],
                                  x_sh_d[:, t * 256:(t + 1) * 256])
                nc.sync.dma_start(x_sl[:, t * 256:(t + 1) * 256],
                                  x_sl_d[:, t * 256:(t + 1) * 256])
                nc.sync.dma_start(w0h[:, t * KO:(t + 1) * KO],
                                  w0h_d[:, t * KO:(t + 1) * KO])
                nc.sync.dma_start(w0l[:, t * KO:(t + 1) * KO],
                                  w0l_d[:, t * KO:(t + 1) * KO])
            for t in range(NSUB):
                nc.sync.dma_start(x_s[:, t * 256:(t + 1) * 256],
                                  x_s_d[:, t * 256:(t + 1) * 256])
                nc.sync.dma_start(w_sb[:, t * KO:(t + 1) * KO],
                                  w_d[:, t * KO:(t + 1) * KO])
            for g in range(3):
                sl = slice(g * 6 * 128, (g + 1) * 6 * 128)
                nc.sync.dma_start(x_ph[:, sl], x_ph_d[:, sl])
                nc.sync.dma_start(x_pl[:, sl], x_pl_d[:, sl])

            x_s_v = x_s.rearrange("p (t b) -> p t b", t=NSUB)
            x_sh_v = x_sh.rearrange("p (t b) -> p t b", t=NSUB)
            x_sl_v = x_sl.rearrange("p (t b) -> p t b", t=NSUB)
            x_ph_v = x_ph.rearrange("p (c t r) -> p c t r", c=2, t=NSUB)
            x_pl_v = x_pl.rearrange("p (c t r) -> p c t r", c=2, t=NSUB)

            for _rep in range(repeat):
              for it in range(ITERS):
                scale = 0.1 if it == 0 else 1.0

                # ---- s partial: 2 b-chunks x 9 K-tiles ----
                s_ps = ps.tile([128, 2 * KO], f32, tag="s_ps")
                for bc in range(2):
                    out = s_ps[:, bc * KO:(bc + 1) * KO]
                    if it == 0:
                        # bf16 hi/lo 3-chain vs host-split W halves
                        for t in range(NSUB):
                            xh = x_sh_v[:, t, bc * 128:(bc + 1) * 128]
                            xl = x_sl_v[:, t, bc * 128:(bc + 1) * 128]
                            wh = w0h[:, t * KO:(t + 1) * KO]
                            wl = w0l[:, t * KO:(t + 1) * KO]
                            nc.tensor.matmul(out, xh, wh,
                                             start=(t == 0), stop=False)
                            nc.tensor.matmul(out, xl, wh,
                                             start=False, stop=False)
                            nc.tensor.matmul(out, xh, wl, start=False,
                                             stop=(t == NSUB - 1))
                    elif it == 1:
                        for t in range(NSUB):
                            nc.tensor.matmul(
                                out, x_s_v[:, t, bc * 128:(bc + 1) * 128],
                                wc[:, t * KO:(t + 1) * KO],
                                start=(t == 0), stop=(t == NSUB - 1))
                    else:
                        for t in range(NSUB):
                            xh = x_sh_v[:, t, bc * 128:(bc + 1) * 128]
                            xl = x_sl_v[:, t, bc * 128:(bc + 1) * 128]
                            wh = wc2h[:, t * KO:(t + 1) * KO]
                            wl = wc2l[:, t * KO:(t + 1) * KO]
                            nc.tensor.matmul(out, xh, wh,
                                             start=(t == 0), stop=False)
                            nc.tensor.matmul(out, xl, wh,
                                             start=False, stop=False)
                            nc.tensor.matmul(out, xh, wl, start=False,
                                             stop=(t == NSUB - 1))
                s_loc = pool.tile([128, 2 * KO], f32, tag="s_loc")
                nc.scalar.copy(s_loc, s_ps)

                ar_in = dram.tile([128, 2 * KO], f32, tag="ar_in")
                nc.sync.dma_start(ar_in[:, :], s_loc[:, :])

                if it == ITERS - 1:
                    # ---- ReduceScatter: keep only our batch shard ----
                    rs_out = dram.tile([16, 2 * KO], f32, tag="rs_out")
                    if collective:
                        nc.gpsimd.collective_compute(
                            "ReduceScatter", ALU.add,
                            replica_groups=[list(range(num_devices))],
                            ins=[ar_in.opt()], outs=[rs_out.opt()])
                        s16 = pool.tile([16, 2 * KO], f32, tag="s16")
                        nc.sync.dma_start(s16[:, :], rs_out[:, :])
                    else:
                        s16 = pool.tile([16, 2 * KO], f32, tag="s16")
                        nc.sync.dma_start(s16[:, :], ar_in[:16, :])
                    sq = pool.tile([16, 2 * KO], f32, tag="f_sq")
                    sg = pool.tile([16, 2 * KO], f32, tag="f_sg")
                    rc = pool.tile([16, 2 * KO], f32, tag="f_rc")
                    v16 = pool.tile([16, 2 * KO], f32, tag="f_v")
                    nc.scalar.activation(sq, s16, ACT.Square, scale=scale)
                    nc.scalar.activation(sg, s16, ACT.Sign)
                    nc.vector.tensor_scalar_add(rc, sq, 1.0)
                    nc.vector.reciprocal_approx_fast(rc, rc)
                    nc.vector.tensor_mul(sq, sq, rc)
                    nc.vector.tensor_mul(v16, sq, sg)
                    nc.sync.dma_start(v_d[:, :], v16[:, :])
                    continue

                ar_out = dram.tile([128, 2 * KO], f32, tag="ar_out",
                                   addr_space="Shared")
                if collective:
                    nc.gpsimd.collective_compute(
                        "AllReduce", ALU.add,
                        replica_groups=[list(range(num_devices))],
                        ins=[ar_in.opt()], outs=[ar_out.opt()])
                else:
                    nc.sync.dma_start(ar_out[:, :], ar_in[:, :])
                s_red = pool.tile([128, 2 * KO], f32, tag="s_red")
                nc.sync.dma_start(s_red[:, :], ar_out[:, :])

                # ---- squash -> v, vh, vl ----
                sq = pool.tile([128, 2 * KO], f32, tag="sq")
                sg = pool.tile([128, 2 * KO], f32, tag="sg")
                rc = pool.tile([128, 2 * KO], f32, tag="rc")
                v_sb = pool.tile([128, 2 * KO], f32, tag="v_sb")
                vh = pool.tile([128, 2 * KO], bf16, tag="vh")
                vl = pool.tile([128, 2 * KO], bf16, tag="vl")
                nc.scalar.activation(sq, s_red, ACT.Square, scale=scale)
                nc.scalar.activation(sg, s_red, ACT.Sign)
                nc.vector.tensor_scalar_add(rc, sq, 1.0)
                nc.vector.reciprocal_approx_fast(rc, rc)
                nc.vector.tensor_mul(sq, sq, rc)
                nc.vector.tensor_mul(v_sb, sq, sg)
                nc.scalar.activation(vh, v_sb, ACT.Copy)
                nc.gpsimd.tensor_sub(vl, v_sb, vh)

                # ---- P-matmul, then one fused W*P pass + one o-reduce ----
                # bank-padded: 3 tiles x 160 per 512-fp32 bank (MM <= 1 bank)
                p_ps = pp.tile([128, 3 * 512], f32, tag="p_ps")
                for t in range(NSUB):
                    off = (t // 3) * 512 + (t % 3) * KO
                    out = p_ps[:, off:off + KO]
                    nc.tensor.matmul(out, x_ph_v[:, 0, t],
                                     vh[:, 0:KO], start=True, stop=False)
                    nc.tensor.matmul(out, x_ph_v[:, 0, t],
                                     vl[:, 0:KO], start=False, stop=False)
                    nc.tensor.matmul(out, x_pl_v[:, 0, t],
                                     vh[:, 0:KO], start=False, stop=False)
                    nc.tensor.matmul(out, x_ph_v[:, 1, t],
                                     vh[:, KO:2 * KO], start=False, stop=False)
                    nc.tensor.matmul(out, x_ph_v[:, 1, t],
                                     vl[:, KO:2 * KO], start=False, stop=False)
                    nc.tensor.matmul(out, x_pl_v[:, 1, t],
                                     vh[:, KO:2 * KO], start=False, stop=True)
                nc.vector.tensor_tensor(
                    out=wp.rearrange("p (b x) -> p b x", b=3),
                    in0=p_ps.rearrange("p (b q) -> p b q", b=3)[:, :, :3 * KO],
                    in1=w_sb.rearrange("p (b x) -> p b x", b=3),
                    op=ALU.mult)
                nc.vector.reduce_sum(
                    q_sb.rearrange("p (t k) -> p t k", t=NSUB),
                    wp.rearrange("p (t k o) -> p t k o", t=NSUB, k=C),
                    axis=AX.X)

                # ---- i-reduce on PE: a = E2^T @ q ----
                a_ps = pa.tile([16, NSUB * C], f32, tag="a_ps")
                nc.tensor.matmul(a_ps[:, :], e2[:, :], q_sb[:, :],
                                 start=True, stop=True)
                if it == 0:
                    nc.vector.tensor_copy(b16, a_ps)
                else:
                    nc.vector.tensor_add(b16, b16, a_ps)

                # ---- softmax over k (scale 1/B in Exp, no max-sub) ----
                e_sb = pool.tile([16, NSUB * C], f32, tag="e_sb")
                sm = pool.tile([16, NSUB], f32, tag="sm")
                c_sb = pool.tile([16, NSUB * C], f32, tag="c_sb")
                nc.scalar.activation(e_sb, b16, ACT.Exp, scale=1.0 / B)
                ev = e_sb.rearrange("p (t k) -> p t k", t=NSUB)
                cv = c_sb.rearrange("p (t k) -> p t k", t=NSUB)
                nc.vector.reduce_sum(sm, ev, axis=AX.X)
                nc.vector.reciprocal(sm, sm)
                nc.vector.tensor_mul(
                    cv, ev, sm.unsqueeze(2).broadcast_to((16, NSUB, C)))

                # ---- c-replicate on PE, then Wc = W * c_rep (gpsimd) ----
                crep_ps = pa.tile([128, NSUB * C], f32, tag="crep_ps")
                nc.tensor.matmul(crep_ps[:, :], erep[:, :], c_sb[:, :],
                                 start=True, stop=True)
                c_rep = pool.tile([128, NSUB * C], f32, tag="c_rep")
                nc.scalar.copy(c_rep, crep_ps)
                wc_out = wc if it == 0 else wp
                nc.gpsimd.tensor_tensor(
                    out=wc_out.rearrange("p (t k o) -> p t k o", t=NSUB, k=C),
                    in0=w_sb.rearrange("p (t k o) -> p t k o", t=NSUB, k=C),
                    in1=c_rep.rearrange("p (t k) -> p t k", t=NSUB)
                        .unsqueeze(3).broadcast_to((128, NSUB, C, O)),
                    op=ALU.mult)
                if it == 1:
                    # split Wc2 into bf16 hi/lo for the final s-matmul
                    nc.gpsimd.tensor_copy(wc2h, wp)
                    nc.gpsimd.tensor_sub(wc2l, wp, wc2h)

    nc.compile()
    return nc


def _host_prep(x, W):
    import ml_dtypes
    bf = ml_dtypes.bfloat16
    x = np.asarray(x, np.float32)
    W0 = np.asarray(W, np.float32)[0]
    p = np.arange(128)
    i_of_p = p // 16
    noff_of_p = p % 16
    E2 = (p[:, None] % 16 == np.arange(16)[None, :]).astype(np.float32)
    Erep = np.ascontiguousarray(E2.T)

    in_maps = []
    for c in range(NCORES):
        n_g = c * NL + (np.arange(NSUB)[:, None] * 16 + noff_of_p[None, :])
        x_g = x[:, n_g, i_of_p[None, :]]                  # [B, 9, 128]
        x_s = np.ascontiguousarray(
            x_g.transpose(2, 1, 0).reshape(128, NSUB * 256))
        x_sh = x_s.astype(bf)
        x_sl = (x_s - x_sh.astype(np.float32)).astype(bf)
        x_p = np.ascontiguousarray(
            x_g.reshape(2, 128, NSUB, 128).transpose(1, 0, 2, 3)
               .reshape(128, 2 * NSUB * 128))
        xh = x_p.astype(bf)
        xl = (x_p - xh.astype(np.float32)).astype(bf)
        w_tmp = W0[n_g, :, :, i_of_p[None, :]]            # [9, 128, C, O]
        w_sb = np.ascontiguousarray(
            w_tmp.transpose(1, 0, 2, 3).reshape(128, NSUB * KO))
        w0h = w_sb.astype(bf)
        w0l = (w_sb - w0h.astype(np.float32)).astype(bf)
        in_maps.append({"x_s": x_s, "x_sh": x_sh, "x_sl": x_sl,
                        "x_ph": xh, "x_pl": xl, "w_s": w_sb,
                        "w0h": w0h, "w0l": w0l, "e2": E2, "erep": Erep})
    return in_maps


def kernel(x, W):
    from concourse import bass_utils

    if "nc" not in _BUILT:
        _BUILT["nc"] = _build_program()
    nc = _BUILT["nc"]

    in_maps = _host_prep(x, W)
    res = bass_utils.run_bass_kernel_spmd(
        nc, in_maps, core_ids=list(range(NCORES)))
    # core r holds v for b in {16r+j} (cols 0:KO) and {128+16r+j} (cols KO:)
    v_full = np.zeros((B, KO), np.float32)
    for c in range(NCORES):
        sh = res.results[c]["v_out"]        # [16, 2*KO]
        v_full[16 * c:16 * c + 16] = sh[:, :KO]
        v_full[128 + 16 * c:128 + 16 * c + 16] = sh[:, KO:]
    return v_full.reshape(B, C, O, 1).astype(np.float32)


if __name__ == "__main__":
    rng = np.random.default_rng(0)
    x = rng.standard_normal((B, N, I), np.float32)
    W = rng.standard_normal((1, N, C, O, I), np.float32)
    out = kernel(x, W)
    print(out.shape, out.dtype, np.abs(out).max())
